# revision 5
# baseline (speedup 1.0000x reference)
"""Affinity-propagate (SPN) Trainium2 Bass kernel, fp16 pipeline.

Computation (per batch element, see reference):
    w = g / conv3x3_ones(|g|)          # gates, [8, H, W], computed once
    d_{k+1} = max_c conv3x3_ones(w_c * d_k)   # 8 iterations

Distribution: pure data parallel, batch element b -> NeuronCore b (8 cores).

Pipeline (per core, H=352 rows as 3 overlapping 128-row tiles):
  - inputs staged fp16 on host: halves HBM traffic and lets every on-chip
    elementwise op run in the DVE 2x/4x fp16 modes.
  - g loaded straight into the w tiles; |g| via DVE tensor_scalar
    bitwise-AND 0x7fff (4x mode); w = g * recip in place.
  - 3x3 conv = tri-band matmul over H (fp16 stationary) x 3 PSUM-accumulated
    W-shifts, 3 chunks of 406 cols per channel; PSUM->SBUF fp16 evacuation
    into a per-tile [128, 8, 1218] prop buffer by ONE ScalarE copy/channel.
  - channel max: 3 DVE ops per tile (stride-2 pair max 8->4->2->1).
  - p = w * d is ONE DVE mult per tile ([128, 8, WB], d broadcast).
  - seam rows between H tiles fixed with 1-row SBUF->SBUF DMAs.
  - final iteration's max writes fp32 staging directly; per-tile output DMA.
"""
from contextlib import ExitStack

import numpy as np

import concourse.bacc as bacc
import concourse.mybir as mybir
import concourse.tile as tile
from concourse.bass_utils import run_bass_kernel_spmd

F32 = mybir.dt.float32
F16 = mybir.dt.float16
U16 = mybir.dt.uint16
ALU = mybir.AluOpType

B, C, H, W = 8, 8, 352, 1216
NCHUNK = 3
CW = 406                        # chunk width; 3 chunks of 406 = 1218 >= W
WB = NCHUNK * CW + 2            # 1220: [0]=pad, 1..1216 data, 1217+ pad
N_ITERS = 8
N_CORES = 8

ROW_BASE = [0, 126, 252]       # first global row of each H tile
ROWS = [128, 128, 100]         # partitions used by each H tile


def _build_nc():
    nc = bacc.Bacc("TRN2", target_bir_lowering=False, debug=False,
                   num_devices=N_CORES)
    g = nc.dram_tensor("g", [C, H, W], F16, kind="ExternalInput").ap()
    d_in = nc.dram_tensor("d", [H, W], F16, kind="ExternalInput").ap()
    band = nc.dram_tensor("band", [128, 128], F16, kind="ExternalInput").ap()
    out = nc.dram_tensor("out", [H, W], F32, kind="ExternalOutput").ap()

    with tile.TileContext(nc) as tc, ExitStack() as ctx:
        pw = ctx.enter_context(tc.tile_pool(name="w", bufs=1))
        pd = ctx.enter_context(tc.tile_pool(name="d", bufs=1))
        pc = ctx.enter_context(tc.tile_pool(name="const", bufs=1))
        pprop = ctx.enter_context(tc.tile_pool(name="prop", bufs=2))
        pp = ctx.enter_context(tc.tile_pool(name="p", bufs=2))
        pm4 = ctx.enter_context(tc.tile_pool(name="m4", bufs=2))
        pm2 = ctx.enter_context(tc.tile_pool(name="m2", bufs=2))
        pa = ctx.enter_context(tc.tile_pool(name="a16", bufs=2))
        pr32 = ctx.enter_context(tc.tile_pool(name="r32", bufs=2))
        pr16 = ctx.enter_context(tc.tile_pool(name="r16", bufs=2))
        po = ctx.enter_context(tc.tile_pool(name="o32", bufs=2))
        psum = ctx.enter_context(tc.tile_pool(name="psum", bufs=2,
                                              space="PSUM"))

        A = pc.tile([128, 128], F16, tag="band", name="bandt")
        nc.sync.dma_start(A[:], band[:])

        wt = [pw.tile([128, C, WB], F16, tag=f"w{t}", name=f"w{t}")
              for t in range(3)]
        dt_ = [pd.tile([128, WB], F16, tag=f"d{t}", name=f"d{t}")
               for t in range(3)]

        g_queues = [nc.sync, nc.scalar]

        # ---- load depth fp16 + zero pads ----
        for t in range(3):
            R, rb = ROWS[t], ROW_BASE[t]
            nc.vector.memset(wt[t][:, :, 0:1], 0.0)
            nc.vector.memset(wt[t][:, :, W + 1:WB], 0.0)
            nc.vector.memset(dt_[t][:, 0:1], 0.0)
            nc.vector.memset(dt_[t][:, W + 1:WB], 0.0)
            nc.gpsimd.dma_start(dt_[t][0:R, 1:W + 1], d_in[rb:rb + R, :])

        # ---- phase 0: w = g / conv3x3_ones(|g|), in place over g ----
        a16_bufs = [pa.tile([128, WB], F16, tag="a16", name=f"a16_{i}")
                    for i in range(2)]
        for buf in a16_bufs:
            nc.vector.memset(buf[:, 0:1], 0.0)
            nc.vector.memset(buf[:, W + 1:WB], 0.0)
        a16_ctr = [0]

        def phase0_pair(pair):
            c0 = 2 * pair
            for t in range(3):
                R, rb = ROWS[t], ROW_BASE[t]
                q = g_queues[(pair * 3 + t) % 2]
                q.dma_start(
                    wt[t][0:R, c0:c0 + 2, 1:W + 1],
                    g[c0:c0 + 2, rb:rb + R, :].rearrange("c p w -> p c w"))
                for c in (c0, c0 + 1):
                    a16 = a16_bufs[a16_ctr[0] % 2]
                    a16_ctr[0] += 1
                    nc.vector.tensor_scalar(
                        a16[0:R, 1:W + 1].bitcast(U16),
                        wt[t][0:R, c, 1:W + 1].bitcast(U16),
                        0x7FFF, None, ALU.bitwise_and)
                    ps = psum.tile([128, NCHUNK, 512], F32, tag="ps",
                                   name="ps")
                    for k in range(NCHUNK):
                        for s in range(3):
                            nc.tensor.matmul(
                                ps[0:R, k, 0:CW], A[0:R, 0:R],
                                a16[0:R, k * CW + s:k * CW + s + CW],
                                start=(s == 0), stop=(s == 2))
                    r32 = pr32.tile([128, NCHUNK, CW], F32, tag="r32",
                                    name="r32")
                    nc.vector.reciprocal_approx_fast(
                        out=r32[0:R, :, :], in_=ps[0:R, :, 0:CW])
                    r16 = pr16.tile([128, NCHUNK * CW], F16, tag="r16",
                                    name="r16")
                    nc.scalar.copy(
                        r16[0:R, :].rearrange("p (a b) -> p a b", a=NCHUNK),
                        r32[0:R, :, :])
                    # w = g * recip, in place (2 of 8 channels on GpSimd)
                    eng = nc.gpsimd if c == 7 else nc.vector
                    eng.tensor_tensor(wt[t][0:R, c, 1:W + 1],
                                      wt[t][0:R, c, 1:W + 1],
                                      r16[0:R, 0:W], ALU.mult)
            # w seam rows for this channel pair
            c1 = c0 + 2
            nc.sync.dma_start(wt[0][127:128, c0:c1, 1:W + 1],
                              wt[1][1:2, c0:c1, 1:W + 1])
            nc.sync.dma_start(wt[1][0:1, c0:c1, 1:W + 1],
                              wt[0][126:127, c0:c1, 1:W + 1])
            nc.sync.dma_start(wt[1][127:128, c0:c1, 1:W + 1],
                              wt[2][1:2, c0:c1, 1:W + 1])
            nc.sync.dma_start(wt[2][0:1, c0:c1, 1:W + 1],
                              wt[1][126:127, c0:c1, 1:W + 1])

        for pair in range(4):
            phase0_pair(pair)

        # ---- one iteration for one tile ----
        outspec = {0: (0, 127), 1: (1, 127), 2: (1, 100)}

        def iter_tile(t, last):
            R = ROWS[t]
            p16 = pp.tile([128, C, WB], F16, tag="p", name="p16")
            dbc = dt_[t][0:R, :].unsqueeze(1).broadcast_to([R, C, WB])
            nc.vector.tensor_mul(p16[0:R, :, :], wt[t][0:R, :, :], dbc)
            prop = pprop.tile([128, C, NCHUNK * CW], F16, tag="prop",
                              name="prop")
            for c in range(C):
                ps = psum.tile([128, NCHUNK, 512], F32, tag="ps", name="ps")
                for kk in range(NCHUNK):
                    for s in range(3):
                        nc.tensor.matmul(
                            ps[0:R, kk, 0:CW], A[0:R, 0:R],
                            p16[0:R, c, kk * CW + s:kk * CW + s + CW],
                            start=(s == 0), stop=(s == 2))
                nc.scalar.copy(
                    prop[0:R, c, :].rearrange("p (a b) -> p a b", a=NCHUNK),
                    ps[0:R, :, 0:CW])
            m4 = pm4.tile([128, 4, W], F16, tag="m4", name="m4")
            nc.vector.tensor_max(m4[0:R, :, :],
                                 prop[0:R, 0:C:2, 0:W],
                                 prop[0:R, 1:C:2, 0:W])
            m2 = pm2.tile([128, 2, W], F16, tag="m2", name="m2")
            nc.vector.tensor_max(m2[0:R, :, :],
                                 m4[0:R, 0:4:2, :], m4[0:R, 1:4:2, :])
            if not last:
                nc.vector.tensor_max(dt_[t][0:R, 1:W + 1],
                                     m2[0:R, 0, :], m2[0:R, 1, :])
            else:
                r0, r1 = outspec[t]
                o32 = po.tile([128, W], F32, tag="o32", name="o32")
                nc.vector.tensor_max(o32[0:R, :],
                                     m2[0:R, 0, :], m2[0:R, 1, :])
                gb = ROW_BASE[t] + r0
                g_queues[t % 2].dma_start(out[gb:gb + (r1 - r0), :],
                                          o32[r0:r1, :])

        def d_seams():
            nc.sync.dma_start(dt_[0][127:128, 1:W + 1], dt_[1][1:2, 1:W + 1])
            nc.sync.dma_start(dt_[1][0:1, 1:W + 1], dt_[0][126:127, 1:W + 1])
            nc.sync.dma_start(dt_[1][127:128, 1:W + 1], dt_[2][1:2, 1:W + 1])
            nc.sync.dma_start(dt_[2][0:1, 1:W + 1], dt_[1][126:127, 1:W + 1])

        for k in range(N_ITERS):
            last = k == N_ITERS - 1
            for t in range(3):
                iter_tile(t, last)
            if not last:
                d_seams()

    nc.compile()
    return nc


def _band_matrix():
    a = np.zeros((128, 128), dtype=np.float16)
    idx = np.arange(128)
    a[idx, idx] = 1.0
    a[idx[:-1], idx[:-1] + 1] = 1.0
    a[idx[1:], idx[1:] - 1] = 1.0
    return a


_NC_CACHE = None


def kernel(guidance: np.ndarray, blur_depth: np.ndarray) -> np.ndarray:
    """Full inputs in, full output out. Shards batch across 8 NeuronCores."""
    global _NC_CACHE
    guidance = np.asarray(guidance)
    blur_depth = np.asarray(blur_depth)
    assert guidance.shape == (B, C, H, W), guidance.shape
    assert blur_depth.shape == (B, 1, H, W), blur_depth.shape
    if _NC_CACHE is None:
        _NC_CACHE = _build_nc()
    nc = _NC_CACHE
    band = _band_matrix()
    g16 = np.ascontiguousarray(guidance.astype(np.float16))
    d16 = np.ascontiguousarray(blur_depth.astype(np.float16))
    in_maps = [
        {"g": g16[b], "d": d16[b, 0], "band": band}
        for b in range(B)
    ]
    res = run_bass_kernel_spmd(nc, in_maps, core_ids=list(range(N_CORES)))
    out = np.stack([res.results[b]["out"] for b in range(B)])[:, None]
    return out.astype(np.float32)
